# revision 34
# baseline (speedup 1.0000x reference)
"""Trainium2 Bass kernel for nn_Autograd4bitQuantLinear (4-bit quant linear).

Computes out = x @ dequant4(qweight, scales, zeros) + bias where
  x:       (4, 2048, 4096) f32
  qweight: (512, 11008)    i32  (8 nibbles packed per int32 along rows)
  scales:  (11008, 1)      f32
  zeros:   (11008, 1)      f32
  bias:    (11008,)        f32
  out:     (4, 2048, 11008) f32

Strategy (tensor-parallel over 8 NeuronCores, column-sharded out_features):
  - Each core owns 1376 output columns; x is replicated.
  - qweight is host-reinterpreted as int16 (pure bit-layout view: each int32
    is two int16 halves, each holding 4 nibbles) and row-replicated 4x so
    SBUF partition p holds the packed word for k-row p. Halves DMA traffic
    vs the int32/8x layout and doubles DVE shift/and throughput.
  - On-device dequant: nib = (q16 >> shamt[p]) & 0xF on DVE, fold
    scale/zero (W = q * s - z), store W bf16 [4096, 1376] SBUF-resident in
    three column groups (one per PSUM n-chunk).
  - x is cast f32->bf16 by SWDGE cast-DMA into DRAM scratch per 512-row
    chunk, then DMA-transposed (xbar) into SBUF as [k, m] tiles. 64 xt
    buffers = 2 chunks fully double-buffered so chunk c+1 transposes
    complete while chunk c computes (no chunk-boundary PE stalls).
  - PE: out[m, n] accumulated over 32 k-tiles in PSUM (bf16 x bf16 -> f32),
    k-outer/n-chunk-inner so 3 consecutive matmuls share the stationary.
  - Epilogue: psum + bias (f32, DVE) -> SBUF -> per-chunk DMA out.
  - Queue split: sync = xbar transposes; scalar = qweight loads (all issued
    BEFORE any output store to avoid head-of-line blocking the unpack
    behind epilogues) + output stores; gpsimd (SWDGE) = cast + broadcasts.
  - Prologue: chunk 0's cast is split in two 256-row halves so the first
    transposes land ~18us in; unpack groups interleave with chunk 0's
    matmul groups so the PE starts ~35us in and stays busy.
"""

import sys

sys.path.insert(0, "/opt/trn_rl_repo")

import ml_dtypes
import numpy as np

import concourse.bass as bass
import concourse.mybir as mybir
from concourse import bacc
from concourse.tile import TileContext
from concourse.tile_rust import add_dep_helper


dt = mybir.dt
AL = mybir.AluOpType

P = 128
IN = 4096  # contraction dim (in_features)
OUT = 11008  # out_features
M_ROWS = 8192  # 4 * 2048
NCORES = 8
NSH = OUT // NCORES  # 1376 output columns per core
KT = IN // P  # 32 k-tiles
M_CHUNK = 512  # rows per x transpose/staging chunk
# n-chunks within the per-core shard; each must fit one PSUM bank (<=512 f32).
# Balanced widths keep every matmul's moving stream well above the ~107ns
# LDWEIGHTS shadow.
N_CHUNKS = ((0, 464), (464, 464), (928, 448))
XT_BUFS = 2  # whole-chunk 3D xt tiles, double-buffered


def build(m_rows=M_ROWS, debug=False):
    """Build + compile the single-core Tile program (SPMD: same on all cores)."""
    assert m_rows % M_CHUNK == 0
    nc = bacc.Bacc(None, target_bir_lowering=False, debug=debug)

    x_d = nc.dram_tensor("x", [m_rows, IN], dt.float32, kind="ExternalInput")
    qw_d = nc.dram_tensor("qw", [IN, NSH], dt.int16, kind="ExternalInput")
    s_d = nc.dram_tensor("scales", [NSH], dt.bfloat16, kind="ExternalInput")
    z_d = nc.dram_tensor("zeros", [NSH], dt.bfloat16, kind="ExternalInput")
    b_d = nc.dram_tensor("bias", [NSH], dt.bfloat16, kind="ExternalInput")
    shamt_d = nc.dram_tensor("shamt", [P, 1], dt.int16, kind="ExternalInput")
    out_d = nc.dram_tensor("out", [m_rows, NSH], dt.float32, kind="ExternalOutput")

    n_mchunks = m_rows // M_CHUNK
    mt_per_chunk = M_CHUNK // P

    with TileContext(nc) as tc:
        with (
            tc.tile_pool(name="singles", bufs=1) as singles,
            tc.tile_pool(name="w", bufs=KT) as wpool,
            tc.tile_pool(name="unpack", bufs=3) as upool,
            tc.tile_pool(name="unpack2", bufs=2) as u2pool,
            tc.tile_pool(name="xsb", bufs=2) as xsbpool,
            tc.tile_pool(name="xt", bufs=XT_BUFS) as xtpool,
            tc.tile_pool(name="osb", bufs=3) as opool,
            tc.tile_pool(name="ps", bufs=2, space="PSUM") as pspool,
        ):
            # ---- constants (sync HWDGE queue: keep gpsimd free for the
            # chunk-0 cast chain, which gates the PE start) ----
            s_rep = singles.tile([P, NSH], dt.bfloat16, tag="s_rep")
            nc.sync.dma_start(out=s_rep[:], in_=s_d[None, :].to_broadcast([P, NSH]))
            z_rep = singles.tile([P, NSH], dt.bfloat16, tag="z_rep")
            nc.sync.dma_start(out=z_rep[:], in_=z_d[None, :].to_broadcast([P, NSH]))
            b_rep = singles.tile([P, NSH], dt.bfloat16, tag="b_rep")
            nc.sync.dma_start(out=b_rep[:], in_=b_d[None, :].to_broadcast([P, NSH]))
            shamt = singles.tile([P, 1], dt.int16, tag="shamt")
            nc.scalar.dma_start(out=shamt[:], in_=shamt_d[:])
            mask = singles.tile([P, 1], dt.int16, tag="mask")
            nc.vector.memset(mask[:], 15)

            # ---- PE warmup: ~112 dummy matmuls on a memset tile fill the
            # otherwise-idle first ~25us (input DMA latency) and trip the
            # HAM activity monitor to full clock before real work arrives.
            warmsb = singles.tile([P, 464], dt.bfloat16, tag="warmsb")
            nc.vector.memset(warmsb[:], 0)
            warmps = pspool.tile([P, 464], dt.float32, tag="warm", name="warm")
            for _ in range(112):
                nc.tensor.matmul(
                    warmps[:], warmsb[:, 0:P], warmsb[:], start=True, stop=True
                )

            # ---- W dequant: three column groups, tiles per (group, k) ----
            wtiles = {}  # (i, k) -> [P, w_i] bf16 tile

            def load_group_qt(i, pool, tagsuf=""):
                """qweight loads for group i (scalar HWDGE queue)."""
                o, wd = N_CHUNKS[i]
                qts = []
                for k in range(KT):
                    qt = pool.tile([P, wd], dt.int16, tag=f"qt{tagsuf}", name="qt")
                    nc.scalar.dma_start(
                        out=qt[:], in_=qw_d[k * P : (k + 1) * P, o : o + wd]
                    )
                    qts.append(qt)
                return qts

            def unpack_group(i, qts, eng, pool, tagsuf=""):
                """Nibble-extract + affine dequant for group i on `eng`
                (DVE or gpsimd; gpsimd runs group 2 in parallel with DVE's
                groups 0/1 so the prologue unpack chain is ~2 groups long)."""
                o, wd = N_CHUNKS[i]
                for k in range(KT):
                    qt = qts[k]
                    # nib = (q16 >> shamt[p]) & 0xF (int16; bitvec can't cast)
                    nib = pool.tile([P, wd], dt.int16, tag=f"nib{tagsuf}", name="nib")
                    eng.scalar_tensor_tensor(
                        nib[:],
                        qt[:],
                        shamt[:, 0:1],
                        mask[:, 0:1].to_broadcast([P, wd]),
                        AL.logical_shift_right,
                        AL.bitwise_and,
                    )
                    ws = pool.tile([P, wd], dt.bfloat16, tag=f"ws{tagsuf}", name="ws")
                    eng.tensor_tensor(
                        ws[:], nib[:], s_rep[:, o : o + wd], AL.mult
                    )
                    wt = wpool.tile([P, wd], dt.bfloat16, tag=f"w{i}", name=f"w{i}_{k}")
                    eng.tensor_tensor(
                        wt[:], ws[:], z_rep[:, o : o + wd], AL.subtract
                    )
                    wtiles[(i, k)] = wt

            def do_mm(ps, xt, mt, k, i):
                nc.tensor.matmul(
                    ps[:],
                    xt[:, k, mt * P : (mt + 1) * P],
                    wtiles[(i, k)][:],
                    start=(k == 0),
                    stop=(k == KT - 1),
                )

            def epilogue(ps, row, i):
                # 3 ob bufs decouple the DVE epilogue FIFO from out-store
                # ring backpressure (2 was not enough under DMA bursts).
                o, wd = N_CHUNKS[i]
                ob = opool.tile([P, wd], dt.float32, tag=f"ob{i}", name=f"ob{i}")
                nc.vector.tensor_tensor(ob[:], ps[:], b_rep[:, o : o + wd], AL.add)
                nc.scalar.dma_start(out=out_d[row : row + P, o : o + wd], in_=ob[:])

            last_xpose = {}  # mc -> last transpose instruction of that chunk

            def load_chunk(mc):
                """x chunk pipeline, no DRAM staging: SWDGE cast-DMA 128-row
                slabs f32(DRAM) -> bf16(SBUF), then SBUF->SBUF xbar
                transposes into a 3D tile [128, KT, M_CHUNK]
                (xt[p, k, m] = x[m, 128k+p]). Slab granularity keeps DMA
                bursts small so concurrent qweight loads aren't starved, and
                skipping the DRAM round-trip cuts HBM traffic ~43%."""
                r0 = mc * M_CHUNK
                xt = xtpool.tile([P, KT, M_CHUNK], dt.bfloat16, tag="xt", name="xt")
                ti = None
                # chunk 0 leads with two 64-row slabs so the very first
                # transpose lands while the DMA rings are still cold.
                slabs = (64, 64) + (P,) * ((M_CHUNK - P) // P) if mc == 0 else (P,) * (M_CHUNK // P)
                a = 0
                for s, rows in enumerate(slabs):
                    xsb = xsbpool.tile([P, IN], dt.bfloat16, tag="xsb", name="xsb")
                    ci = nc.gpsimd.dma_start(
                        out=xsb[0:rows, :], in_=x_d[r0 + a : r0 + a + rows, :]
                    )
                    # throttle the cast chain: without the explicit dep all
                    # casts flood the SDMA rings at t=0 and starve chunk 0.
                    if s == 0 and mc - 2 in last_xpose:
                        add_dep_helper(
                            ci.ins,
                            last_xpose[mc - 2].ins,
                            sync=True,
                            reason="throttle x cast chain",
                        )
                    ti = nc.sync.dma_start(
                        out=xt[:, :, a : a + rows],
                        in_=xsb[0:rows, :],
                        transpose=True,
                    )
                    a += rows
                last_xpose[mc] = ti
                return xt

            # ---- prologue: chunks 0+1 processed n-chunk-group-major so the
            # PE has ~2x the work per unpack group and its epilogues land in
            # the DVE FIFO between group unpacks (freeing PSUM promptly).
            # All qweight loads for g0/g1 are issued on the scalar queue
            # before any output store so the unpack is never head-of-line
            # blocked; g2's loads trickle in behind g1's unpack.
            qts0 = load_group_qt(0, upool)
            unpack_group(0, qts0, nc.vector, upool)
            xts0 = load_chunk(0)
            xts1 = load_chunk(1)
            qts1 = load_group_qt(1, upool)

            def phase(i):
                for gmt in range(2 * mt_per_chunk):
                    xt = xts0 if gmt < mt_per_chunk else xts1
                    mt = gmt % mt_per_chunk
                    ps = pspool.tile(
                        [P, N_CHUNKS[i][1]], dt.float32,
                        tag=f"ps{i}", name=f"ps{i}",
                    )
                    for k in range(KT):
                        do_mm(ps, xt, mt, k, i)
                    epilogue(ps, gmt * P, i)

            phase(0)
            unpack_group(1, qts1, nc.vector, upool)
            qts2 = load_group_qt(2, u2pool, "2")
            phase(1)
            unpack_group(2, qts2, nc.vector, u2pool, "2")
            phase(2)

            # ---- steady state ----
            for mc in range(2, n_mchunks):
                xts = load_chunk(mc)
                for mt in range(mt_per_chunk):
                    pss = [
                        pspool.tile(
                            [P, wd], dt.float32, tag=f"ps{i}", name=f"ps{i}"
                        )
                        for i, (o, wd) in enumerate(N_CHUNKS)
                    ]
                    for k in range(KT):
                        for i in range(len(N_CHUNKS)):
                            do_mm(pss[i], xts, mt, k, i)
                    for i in range(len(N_CHUNKS)):
                        epilogue(pss[i], mc * M_CHUNK + mt * P, i)

    nc.compile()
    return nc


_SHAMT16 = (4 * (np.arange(P, dtype=np.int16) % 4)).reshape(P, 1)


def _qw_int16(qw_slice):
    """Reinterpret packed int32 nibbles as int16 rows (layout-only) and
    row-replicate 4x so SBUF partition p holds the packed word for k-row p."""
    a = np.ascontiguousarray(qw_slice)  # [512, nsh] int32
    rows, cols = a.shape
    a16 = (
        a.view(np.int16)
        .reshape(rows, cols, 2)
        .transpose(0, 2, 1)
        .reshape(rows * 2, cols)
    )  # row 2r = low 4 nibbles of packed row r, 2r+1 = high 4
    return np.ascontiguousarray(np.repeat(a16, 4, axis=0))


def make_in_maps(x2d, qweight, scales, zeros, bias):
    """Per-core input maps (host-side sharding / layout prep only)."""
    in_maps = []
    for c in range(NCORES):
        sl = slice(c * NSH, (c + 1) * NSH)
        in_maps.append(
            {
                "x": x2d,
                "qw": _qw_int16(qweight[:, sl]),
                "scales": np.ascontiguousarray(scales[sl, 0]).astype(
                    ml_dtypes.bfloat16
                ),
                "zeros": np.ascontiguousarray(zeros[sl, 0]).astype(
                    ml_dtypes.bfloat16
                ),
                "bias": np.ascontiguousarray(bias[sl]).astype(ml_dtypes.bfloat16),
                "shamt": _SHAMT16,
            }
        )
    return in_maps


_NC_CACHE = {}


def _get_nc(m_rows):
    if m_rows not in _NC_CACHE:
        _NC_CACHE[m_rows] = build(m_rows)
    return _NC_CACHE[m_rows]


def run_spmd(x2d, qweight, scales, zeros, bias, trace=False, **kwargs):
    """Run on the 8 NeuronCores; returns (out2d [8192, 11008] f32, results)."""
    from concourse.bass_utils import run_bass_kernel_spmd

    m_rows = x2d.shape[0]
    nc = _get_nc(m_rows)
    in_maps = make_in_maps(x2d, qweight, scales, zeros, bias)
    res = run_bass_kernel_spmd(
        nc, in_maps, list(range(NCORES)), trace=trace, **kwargs
    )
    outs = [res.results[c]["out"] for c in range(NCORES)]
    out2d = np.concatenate(outs, axis=1)
    return out2d, res


def kernel(x, qweight, scales, zeros, bias):
    x = np.asarray(x, dtype=np.float32)
    qweight = np.asarray(qweight, dtype=np.int32)
    scales = np.asarray(scales, dtype=np.float32)
    zeros = np.asarray(zeros, dtype=np.float32)
    bias = np.asarray(bias, dtype=np.float32)

    b, s, k_in = x.shape
    x2d = np.ascontiguousarray(x.reshape(b * s, k_in))
    out2d, _ = run_spmd(x2d, qweight, scales, zeros, bias)
    return out2d.reshape(b, s, OUT)


# revision 36
# speedup vs baseline: 1.0005x; 1.0005x over previous
"""Trainium2 Bass kernel for nn_Autograd4bitQuantLinear (4-bit quant linear).

Computes out = x @ dequant4(qweight, scales, zeros) + bias where
  x:       (4, 2048, 4096) f32
  qweight: (512, 11008)    i32  (8 nibbles packed per int32 along rows)
  scales:  (11008, 1)      f32
  zeros:   (11008, 1)      f32
  bias:    (11008,)        f32
  out:     (4, 2048, 11008) f32

Strategy (tensor-parallel over 8 NeuronCores, column-sharded out_features):
  - Each core owns 1376 output columns; x is replicated.
  - qweight is host-reinterpreted as int16 (pure bit-layout view: each int32
    is two int16 halves, each holding 4 nibbles) and row-replicated 4x so
    SBUF partition p holds the packed word for k-row p. Halves DMA traffic
    vs the int32/8x layout and doubles DVE shift/and throughput.
  - On-device dequant: nib = (q16 >> shamt[p]) & 0xF on DVE, fold
    scale/zero (W = q * s - z, s/z in bf16), store W bf16 [4096, 1376]
    SBUF-resident in three balanced column groups (464/464/448, one per
    PSUM n-chunk; widths keep every matmul's moving stream well above the
    ~107ns LDWEIGHTS shadow).
  - x pipeline with no DRAM staging: SWDGE cast-DMA 128-row slabs
    f32(DRAM) -> bf16(SBUF), then SBUF->SBUF xbar transposes into 3D tiles
    [128, KT, 512] (one DMA_TRANSPOSE per slab — per-instruction issue cost
    ~1.3us makes fine-grained transposes a sync-queue bottleneck).
    Double-buffered whole-chunk xt tiles; skipping the DRAM round-trip cuts
    HBM traffic ~43% so out-stores/qweight loads never starve.
  - PE: out[m, n] accumulated over 32 k-tiles in PSUM (bf16 x bf16 -> f32),
    k-outer/n-chunk-inner so 3 consecutive matmuls share the stationary.
  - Epilogue: psum + bias (DVE) -> SBUF (3 bufs so out-store ring
    backpressure never blocks the DVE FIFO) -> scalar-queue DMA out.
  - Queue split: sync = xbar transposes + constant broadcasts; scalar =
    qweight loads (issued before any output store to avoid head-of-line
    blocking the unpack behind epilogues) + output stores; gpsimd (SWDGE) =
    x cast slabs only.
  - Prologue: chunks 0+1 are processed n-chunk-group-major (phases) so the
    PE has 2 chunks of work per unpack group and each phase's epilogues
    land in the DVE FIFO between group unpacks, freeing PSUM promptly.
    Chunk 0 leads with two 64-row slabs so the first transpose lands while
    the DMA rings are still cold-slow.
"""

import sys

sys.path.insert(0, "/opt/trn_rl_repo")

import ml_dtypes
import numpy as np

import concourse.bass as bass
import concourse.mybir as mybir
from concourse import bacc
from concourse.tile import TileContext
from concourse.tile_rust import add_dep_helper


dt = mybir.dt
AL = mybir.AluOpType

P = 128
IN = 4096  # contraction dim (in_features)
OUT = 11008  # out_features
M_ROWS = 8192  # 4 * 2048
NCORES = 8
NSH = OUT // NCORES  # 1376 output columns per core
KT = IN // P  # 32 k-tiles
M_CHUNK = 512  # rows per x transpose/staging chunk
# n-chunks within the per-core shard; each must fit one PSUM bank (<=512 f32).
# Balanced widths keep every matmul's moving stream well above the ~107ns
# LDWEIGHTS shadow.
N_CHUNKS = ((0, 464), (464, 464), (928, 448))
XT_BUFS = 2  # whole-chunk 3D xt tiles, double-buffered


def build(m_rows=M_ROWS, debug=False):
    """Build + compile the single-core Tile program (SPMD: same on all cores)."""
    assert m_rows % M_CHUNK == 0
    nc = bacc.Bacc(None, target_bir_lowering=False, debug=debug)

    x_d = nc.dram_tensor("x", [m_rows, IN], dt.float32, kind="ExternalInput")
    qw_d = nc.dram_tensor("qw", [IN, NSH], dt.int16, kind="ExternalInput")
    s_d = nc.dram_tensor("scales", [NSH], dt.bfloat16, kind="ExternalInput")
    z_d = nc.dram_tensor("zeros", [NSH], dt.bfloat16, kind="ExternalInput")
    b_d = nc.dram_tensor("bias", [NSH], dt.bfloat16, kind="ExternalInput")
    shamt_d = nc.dram_tensor("shamt", [P, 1], dt.int16, kind="ExternalInput")
    out_d = nc.dram_tensor("out", [m_rows, NSH], dt.float32, kind="ExternalOutput")

    n_mchunks = m_rows // M_CHUNK
    mt_per_chunk = M_CHUNK // P

    with TileContext(nc) as tc:
        with (
            tc.tile_pool(name="singles", bufs=1) as singles,
            tc.tile_pool(name="w", bufs=KT) as wpool,
            tc.tile_pool(name="unpack", bufs=3) as upool,
            tc.tile_pool(name="unpack2", bufs=2) as u2pool,
            tc.tile_pool(name="xsb", bufs=2) as xsbpool,
            tc.tile_pool(name="xt", bufs=XT_BUFS) as xtpool,
            tc.tile_pool(name="osb", bufs=3) as opool,
            tc.tile_pool(name="ps", bufs=2, space="PSUM") as pspool,
        ):
            # ---- constants (sync HWDGE queue: keep gpsimd free for the
            # chunk-0 cast chain, which gates the PE start) ----
            s_rep = singles.tile([P, NSH], dt.bfloat16, tag="s_rep")
            nc.sync.dma_start(out=s_rep[:], in_=s_d[None, :].to_broadcast([P, NSH]))
            z_rep = singles.tile([P, NSH], dt.bfloat16, tag="z_rep")
            nc.sync.dma_start(out=z_rep[:], in_=z_d[None, :].to_broadcast([P, NSH]))
            b_rep = singles.tile([P, NSH], dt.bfloat16, tag="b_rep")
            nc.sync.dma_start(out=b_rep[:], in_=b_d[None, :].to_broadcast([P, NSH]))
            shamt = singles.tile([P, 1], dt.int16, tag="shamt")
            nc.scalar.dma_start(out=shamt[:], in_=shamt_d[:])
            mask = singles.tile([P, 1], dt.int16, tag="mask")
            nc.vector.memset(mask[:], 15)

            # ---- W dequant: three column groups, tiles per (group, k) ----
            wtiles = {}  # (i, k) -> [P, w_i] bf16 tile

            def load_group_qt(i, pool, tagsuf=""):
                """qweight loads for group i (scalar HWDGE queue)."""
                o, wd = N_CHUNKS[i]
                qts = []
                for k in range(KT):
                    qt = pool.tile([P, wd], dt.int16, tag=f"qt{tagsuf}", name="qt")
                    nc.scalar.dma_start(
                        out=qt[:], in_=qw_d[k * P : (k + 1) * P, o : o + wd]
                    )
                    qts.append(qt)
                return qts

            def unpack_group(i, qts, eng, pool, tagsuf=""):
                """Nibble-extract + affine dequant for group i on `eng`
                (DVE or gpsimd; gpsimd runs group 2 in parallel with DVE's
                groups 0/1 so the prologue unpack chain is ~2 groups long)."""
                o, wd = N_CHUNKS[i]
                for k in range(KT):
                    qt = qts[k]
                    # nib = (q16 >> shamt[p]) & 0xF (int16; bitvec can't cast)
                    nib = pool.tile([P, wd], dt.int16, tag=f"nib{tagsuf}", name="nib")
                    eng.scalar_tensor_tensor(
                        nib[:],
                        qt[:],
                        shamt[:, 0:1],
                        mask[:, 0:1].to_broadcast([P, wd]),
                        AL.logical_shift_right,
                        AL.bitwise_and,
                    )
                    ws = pool.tile([P, wd], dt.bfloat16, tag=f"ws{tagsuf}", name="ws")
                    eng.tensor_tensor(
                        ws[:], nib[:], s_rep[:, o : o + wd], AL.mult
                    )
                    wt = wpool.tile([P, wd], dt.bfloat16, tag=f"w{i}", name=f"w{i}_{k}")
                    eng.tensor_tensor(
                        wt[:], ws[:], z_rep[:, o : o + wd], AL.subtract
                    )
                    wtiles[(i, k)] = wt

            def do_mm(ps, xt, mt, k, i):
                nc.tensor.matmul(
                    ps[:],
                    xt[:, k, mt * P : (mt + 1) * P],
                    wtiles[(i, k)][:],
                    start=(k == 0),
                    stop=(k == KT - 1),
                )

            def epilogue(ps, row, i):
                # 3 ob bufs decouple the DVE epilogue FIFO from out-store
                # ring backpressure (2 was not enough under DMA bursts).
                o, wd = N_CHUNKS[i]
                ob = opool.tile([P, wd], dt.float32, tag=f"ob{i}", name=f"ob{i}")
                nc.vector.tensor_tensor(ob[:], ps[:], b_rep[:, o : o + wd], AL.add)
                nc.scalar.dma_start(out=out_d[row : row + P, o : o + wd], in_=ob[:])

            last_xpose = {}  # mc -> last transpose instruction of that chunk

            def load_chunk(mc):
                """x chunk pipeline, no DRAM staging: SWDGE cast-DMA 128-row
                slabs f32(DRAM) -> bf16(SBUF), then SBUF->SBUF xbar
                transposes into a 3D tile [128, KT, M_CHUNK]
                (xt[p, k, m] = x[m, 128k+p]). Slab granularity keeps DMA
                bursts small so concurrent qweight loads aren't starved, and
                skipping the DRAM round-trip cuts HBM traffic ~43%."""
                r0 = mc * M_CHUNK
                xt = xtpool.tile([P, KT, M_CHUNK], dt.bfloat16, tag="xt", name="xt")
                ti = None
                # chunk 0 leads with two 64-row slabs so the very first
                # transpose lands while the DMA rings are still cold.
                slabs = (64, 64) + (P,) * ((M_CHUNK - P) // P) if mc == 0 else (P,) * (M_CHUNK // P)
                a = 0
                for s, rows in enumerate(slabs):
                    xsb = xsbpool.tile([P, IN], dt.bfloat16, tag="xsb", name="xsb")
                    ci = nc.gpsimd.dma_start(
                        out=xsb[0:rows, :], in_=x_d[r0 + a : r0 + a + rows, :]
                    )
                    # throttle the cast chain: without the explicit dep all
                    # casts flood the SDMA rings at t=0 and starve chunk 0.
                    if s == 0 and mc - 2 in last_xpose:
                        add_dep_helper(
                            ci.ins,
                            last_xpose[mc - 2].ins,
                            sync=True,
                            reason="throttle x cast chain",
                        )
                    ti = nc.sync.dma_start(
                        out=xt[:, :, a : a + rows],
                        in_=xsb[0:rows, :],
                        transpose=True,
                    )
                    a += rows
                last_xpose[mc] = ti
                return xt

            # ---- prologue: chunks 0+1 processed n-chunk-group-major so the
            # PE has ~2x the work per unpack group and its epilogues land in
            # the DVE FIFO between group unpacks (freeing PSUM promptly).
            # All qweight loads for g0/g1 are issued on the scalar queue
            # before any output store so the unpack is never head-of-line
            # blocked; g2's loads trickle in behind g1's unpack.
            qts0 = load_group_qt(0, upool)
            unpack_group(0, qts0, nc.vector, upool)
            xts0 = load_chunk(0)
            xts1 = load_chunk(1)
            qts1 = load_group_qt(1, upool)

            def phase(i):
                for gmt in range(2 * mt_per_chunk):
                    xt = xts0 if gmt < mt_per_chunk else xts1
                    mt = gmt % mt_per_chunk
                    ps = pspool.tile(
                        [P, N_CHUNKS[i][1]], dt.float32,
                        tag=f"ps{i}", name=f"ps{i}",
                    )
                    for k in range(KT):
                        do_mm(ps, xt, mt, k, i)
                    epilogue(ps, gmt * P, i)

            phase(0)
            unpack_group(1, qts1, nc.vector, upool)
            qts2 = load_group_qt(2, u2pool, "2")
            phase(1)
            unpack_group(2, qts2, nc.vector, u2pool, "2")
            phase(2)

            # ---- steady state ----
            for mc in range(2, n_mchunks):
                xts = load_chunk(mc)
                for mt in range(mt_per_chunk):
                    pss = [
                        pspool.tile(
                            [P, wd], dt.float32, tag=f"ps{i}", name=f"ps{i}"
                        )
                        for i, (o, wd) in enumerate(N_CHUNKS)
                    ]
                    for k in range(KT):
                        for i in range(len(N_CHUNKS)):
                            do_mm(pss[i], xts, mt, k, i)
                    for i in range(len(N_CHUNKS)):
                        epilogue(pss[i], mc * M_CHUNK + mt * P, i)

    nc.compile()
    return nc


_SHAMT16 = (4 * (np.arange(P, dtype=np.int16) % 4)).reshape(P, 1)


def _qw_int16(qw_slice):
    """Reinterpret packed int32 nibbles as int16 rows (layout-only) and
    row-replicate 4x so SBUF partition p holds the packed word for k-row p."""
    a = np.ascontiguousarray(qw_slice)  # [512, nsh] int32
    rows, cols = a.shape
    a16 = (
        a.view(np.int16)
        .reshape(rows, cols, 2)
        .transpose(0, 2, 1)
        .reshape(rows * 2, cols)
    )  # row 2r = low 4 nibbles of packed row r, 2r+1 = high 4
    return np.ascontiguousarray(np.repeat(a16, 4, axis=0))


def make_in_maps(x2d, qweight, scales, zeros, bias):
    """Per-core input maps (host-side sharding / layout prep only)."""
    in_maps = []
    for c in range(NCORES):
        sl = slice(c * NSH, (c + 1) * NSH)
        in_maps.append(
            {
                "x": x2d,
                "qw": _qw_int16(qweight[:, sl]),
                "scales": np.ascontiguousarray(scales[sl, 0]).astype(
                    ml_dtypes.bfloat16
                ),
                "zeros": np.ascontiguousarray(zeros[sl, 0]).astype(
                    ml_dtypes.bfloat16
                ),
                "bias": np.ascontiguousarray(bias[sl]).astype(ml_dtypes.bfloat16),
                "shamt": _SHAMT16,
            }
        )
    return in_maps


_NC_CACHE = {}


def _get_nc(m_rows):
    if m_rows not in _NC_CACHE:
        _NC_CACHE[m_rows] = build(m_rows)
    return _NC_CACHE[m_rows]


def run_spmd(x2d, qweight, scales, zeros, bias, trace=False, **kwargs):
    """Run on the 8 NeuronCores; returns (out2d [8192, 11008] f32, results)."""
    from concourse.bass_utils import run_bass_kernel_spmd

    m_rows = x2d.shape[0]
    nc = _get_nc(m_rows)
    in_maps = make_in_maps(x2d, qweight, scales, zeros, bias)
    res = run_bass_kernel_spmd(
        nc, in_maps, list(range(NCORES)), trace=trace, **kwargs
    )
    outs = [res.results[c]["out"] for c in range(NCORES)]
    out2d = np.concatenate(outs, axis=1)
    return out2d, res


def kernel(x, qweight, scales, zeros, bias):
    x = np.asarray(x, dtype=np.float32)
    qweight = np.asarray(qweight, dtype=np.int32)
    scales = np.asarray(scales, dtype=np.float32)
    zeros = np.asarray(zeros, dtype=np.float32)
    bias = np.asarray(bias, dtype=np.float32)

    b, s, k_in = x.shape
    x2d = np.ascontiguousarray(x.reshape(b * s, k_in))
    out2d, _ = run_spmd(x2d, qweight, scales, zeros, bias)
    return out2d.reshape(b, s, OUT)


# revision 43
# speedup vs baseline: 1.1896x; 1.1890x over previous
"""Trainium2 Bass kernel for nn_Autograd4bitQuantLinear (4-bit quant linear).

Computes out = x @ dequant4(qweight, scales, zeros) + bias where
  x:       (4, 2048, 4096) f32
  qweight: (512, 11008)    i32  (8 nibbles packed per int32 along rows)
  scales:  (11008, 1)      f32
  zeros:   (11008, 1)      f32
  bias:    (11008,)        f32
  out:     (4, 2048, 11008) f32

Strategy (tensor-parallel over 8 NeuronCores, column-sharded out_features):
  - Each core owns 1376 output columns; x is replicated.
  - qweight is host-reinterpreted as int16 (pure bit-layout view: each int32
    is two int16 halves, each holding 4 nibbles) and row-replicated 4x so
    SBUF partition p holds the packed word for k-row p. Halves DMA traffic
    vs the int32/8x layout and doubles DVE shift/and throughput.
  - On-device dequant: nib = (q16 >> shamt[p]) & 0xF on DVE, fold
    scale/zero (W = q * s - z, s/z in bf16), store W bf16 [4096, 1376]
    SBUF-resident in three balanced column groups (464/464/448, one per
    PSUM n-chunk; widths keep every matmul's moving stream well above the
    ~107ns LDWEIGHTS shadow).
  - x pipeline with no DRAM staging: SWDGE cast-DMA 128-row slabs
    f32(DRAM) -> bf16(SBUF), then SBUF->SBUF xbar transposes into 3D tiles
    [128, KT, 512] (one DMA_TRANSPOSE per slab — per-instruction issue cost
    ~1.3us makes fine-grained transposes a sync-queue bottleneck).
    Double-buffered whole-chunk xt tiles; skipping the DRAM round-trip cuts
    HBM traffic ~43% so out-stores/qweight loads never starve.
  - PE: out[m, n] accumulated over 32 k-tiles in PSUM (bf16 x bf16 -> f32),
    k-outer/n-chunk-inner so 3 consecutive matmuls share the stationary.
  - Epilogue: psum + bias (DVE) -> SBUF (3 bufs so out-store ring
    backpressure never blocks the DVE FIFO) -> scalar-queue DMA out.
  - Queue split: sync = xbar transposes + constant broadcasts; scalar =
    qweight loads (issued before any output store to avoid head-of-line
    blocking the unpack behind epilogues) + output stores; gpsimd (SWDGE) =
    x cast slabs only.
  - Prologue: chunks 0+1 are processed n-chunk-group-major (phases) so the
    PE has 2 chunks of work per unpack group and each phase's epilogues
    land in the DVE FIFO between group unpacks, freeing PSUM promptly.
    Chunk 0 leads with two 64-row slabs so the first transpose lands while
    the DMA rings are still cold-slow.
"""

import sys

sys.path.insert(0, "/opt/trn_rl_repo")

import ml_dtypes
import numpy as np

import concourse.bass as bass
import concourse.mybir as mybir
from concourse import bacc
from concourse.tile import TileContext
from concourse.tile_rust import add_dep_helper


dt = mybir.dt
AL = mybir.AluOpType

P = 128
IN = 4096  # contraction dim (in_features)
OUT = 11008  # out_features
M_ROWS = 8192  # 4 * 2048
NCORES = 8
NSH = OUT // NCORES  # 1376 output columns per core
KT = IN // P  # 32 k-tiles
M_CHUNK = 512  # rows per x transpose/staging chunk
# n-chunks within the per-core shard; each must fit one PSUM bank (<=512 f32).
# Balanced widths keep every matmul's moving stream well above the ~107ns
# LDWEIGHTS shadow.
N_CHUNKS = ((0, 464), (464, 464), (928, 448))
XT_BUFS = 2  # whole-chunk 3D xt tiles, double-buffered


def build(m_rows=M_ROWS, debug=False):
    """Build + compile the single-core Tile program (SPMD: same on all cores)."""
    assert m_rows % M_CHUNK == 0
    nc = bacc.Bacc(None, target_bir_lowering=False, debug=debug)

    x_d = nc.dram_tensor("x", [m_rows, IN], dt.float32, kind="ExternalInput")
    qw_d = nc.dram_tensor("qw", [IN, NSH], dt.int16, kind="ExternalInput")
    s_d = nc.dram_tensor("scales", [NSH], dt.bfloat16, kind="ExternalInput")
    z_d = nc.dram_tensor("zeros", [NSH], dt.bfloat16, kind="ExternalInput")
    b_d = nc.dram_tensor("bias", [NSH], dt.bfloat16, kind="ExternalInput")
    shamt_d = nc.dram_tensor("shamt", [P, 1], dt.int16, kind="ExternalInput")
    out_d = nc.dram_tensor("out", [m_rows, NSH], dt.float32, kind="ExternalOutput")

    n_mchunks = m_rows // M_CHUNK
    mt_per_chunk = M_CHUNK // P

    with TileContext(nc) as tc:
        with (
            tc.tile_pool(name="singles", bufs=1) as singles,
            tc.tile_pool(name="w", bufs=KT // 2) as wpool,
            tc.tile_pool(name="unpack", bufs=2) as upool,
            tc.tile_pool(name="unpack2", bufs=2) as u2pool,
            tc.tile_pool(name="xsb", bufs=1) as xsbpool,
            tc.tile_pool(name="xt", bufs=XT_BUFS) as xtpool,
            tc.tile_pool(name="osb", bufs=3) as opool,
            tc.tile_pool(name="ps", bufs=2, space="PSUM") as pspool,
        ):
            # ---- constants (sync HWDGE queue: keep gpsimd free for the
            # chunk-0 cast chain, which gates the PE start) ----
            s_rep = singles.tile([P, NSH], dt.bfloat16, tag="s_rep")
            nc.sync.dma_start(out=s_rep[:], in_=s_d[None, :].to_broadcast([P, NSH]))
            z_rep = singles.tile([P, NSH], dt.bfloat16, tag="z_rep")
            nc.sync.dma_start(out=z_rep[:], in_=z_d[None, :].to_broadcast([P, NSH]))
            b_rep = singles.tile([P, NSH], dt.bfloat16, tag="b_rep")
            nc.sync.dma_start(out=b_rep[:], in_=b_d[None, :].to_broadcast([P, NSH]))
            shamt = singles.tile([P, 1], dt.int16, tag="shamt")
            nc.scalar.dma_start(out=shamt[:], in_=shamt_d[:])
            mask = singles.tile([P, 1], dt.int16, tag="mask")
            nc.vector.memset(mask[:], 15)

            # ---- W dequant: three column groups, tiles per (group, k) ----
            wtiles = {}  # (i, k) -> [P, w_i] bf16 tile

            def load_group_qt(i, pool, tagsuf=""):
                """qweight loads for group i (scalar HWDGE queue), two
                k-tiles per 3D [P, 2, wd] tile so each DVE unpack op covers
                a k-pair (halves per-op fixed overhead; the shift amount
                depends only on the partition, so a fused op stays correct)."""
                o, wd = N_CHUNKS[i]
                qts = []
                for kp in range(KT // 2):
                    qt = pool.tile([P, 2, wd], dt.int16, tag=f"qt{tagsuf}", name="qt")
                    for h in range(2):
                        k = 2 * kp + h
                        nc.scalar.dma_start(
                            out=qt[:, h, :],
                            in_=qw_d[k * P : (k + 1) * P, o : o + wd],
                        )
                    qts.append(qt)
                return qts

            def unpack_group(i, qts, eng, pool, tagsuf=""):
                """Nibble-extract + affine dequant for group i on `eng`,
                one op per k-pair."""
                o, wd = N_CHUNKS[i]
                for kp in range(KT // 2):
                    qt = qts[kp]
                    # nib = (q16 >> shamt[p]) & 0xF (int16; bitvec can't cast)
                    nib = pool.tile([P, 2, wd], dt.int16, tag=f"nib{tagsuf}", name="nib")
                    eng.scalar_tensor_tensor(
                        nib[:],
                        qt[:],
                        shamt[:, 0:1],
                        mask[:, 0:1].to_broadcast([P, 2, wd]),
                        AL.logical_shift_right,
                        AL.bitwise_and,
                    )
                    wt = wpool.tile([P, 2, wd], dt.bfloat16, tag=f"w{i}", name=f"w{i}_{kp}")
                    eng.tensor_tensor(
                        wt[:], nib[:],
                        s_rep[:, None, o : o + wd].to_broadcast([P, 2, wd]),
                        AL.mult,
                    )
                    # in-place subtract saves the ws intermediate (SBUF)
                    eng.tensor_tensor(
                        wt[:], wt[:],
                        z_rep[:, None, o : o + wd].to_broadcast([P, 2, wd]),
                        AL.subtract,
                    )
                    wtiles[(i, 2 * kp)] = wt[:, 0, :]
                    wtiles[(i, 2 * kp + 1)] = wt[:, 1, :]

            def do_mm(ps, xt, mt, k, i):
                nc.tensor.matmul(
                    ps[:],
                    xt[:, k, mt * P : (mt + 1) * P],
                    wtiles[(i, k)],
                    start=(k == 0),
                    stop=(k == KT - 1),
                )

            def epilogue(ps, row, i):
                # 3 ob bufs decouple the DVE epilogue FIFO from out-store
                # ring backpressure (2 was not enough under DMA bursts).
                o, wd = N_CHUNKS[i]
                ob = opool.tile([P, wd], dt.float32, tag=f"ob{i}", name=f"ob{i}")
                nc.vector.tensor_tensor(ob[:], ps[:], b_rep[:, o : o + wd], AL.add)
                nc.scalar.dma_start(out=out_d[row : row + P, o : o + wd], in_=ob[:])

            last_xpose = {}  # mc -> last transpose instruction of that chunk

            def load_chunk(mc):
                """x chunk pipeline, no DRAM staging: SWDGE cast-DMA 128-row
                slabs f32(DRAM) -> bf16(SBUF), then SBUF->SBUF xbar
                transposes into a 3D tile [128, KT, M_CHUNK]
                (xt[p, k, m] = x[m, 128k+p]). Slab granularity keeps DMA
                bursts small so concurrent qweight loads aren't starved, and
                skipping the DRAM round-trip cuts HBM traffic ~43%."""
                r0 = mc * M_CHUNK
                xt = xtpool.tile([P, KT, M_CHUNK], dt.bfloat16, tag="xt", name="xt")
                ti = None
                # chunk 0 leads with two 64-row slabs so the very first
                # transpose lands while the DMA rings are still cold.
                slabs = (64, 64) + (P,) * ((M_CHUNK - P) // P) if mc == 0 else (P,) * (M_CHUNK // P)
                a = 0
                for s, rows in enumerate(slabs):
                    xsb = xsbpool.tile([P, IN], dt.bfloat16, tag="xsb", name="xsb")
                    ci = nc.gpsimd.dma_start(
                        out=xsb[0:rows, :], in_=x_d[r0 + a : r0 + a + rows, :]
                    )
                    # throttle the cast chain: without the explicit dep all
                    # casts flood the SDMA rings at t=0 and starve chunk 0.
                    if s == 0 and mc - 2 in last_xpose:
                        add_dep_helper(
                            ci.ins,
                            last_xpose[mc - 2].ins,
                            sync=True,
                            reason="throttle x cast chain",
                        )
                    ti = nc.sync.dma_start(
                        out=xt[:, :, a : a + rows],
                        in_=xsb[0:rows, :],
                        transpose=True,
                    )
                    a += rows
                last_xpose[mc] = ti
                return xt

            # ---- prologue: chunks 0+1 processed n-chunk-group-major so the
            # PE has ~2x the work per unpack group and its epilogues land in
            # the DVE FIFO between group unpacks (freeing PSUM promptly).
            # All qweight loads for g0/g1 are issued on the scalar queue
            # before any output store so the unpack is never head-of-line
            # blocked; g2's loads trickle in behind g1's unpack.
            qts0 = load_group_qt(0, upool)
            unpack_group(0, qts0, nc.vector, upool)
            xts0 = load_chunk(0)
            xts1 = load_chunk(1)
            qts1 = load_group_qt(1, upool)

            def phase(i):
                # gmt-pair k-outer sweeps: both ps bufs live per sweep so
                # each freshly-unpacked W tile feeds two matmuls while the
                # PE drips just behind the DVE unpack chain.
                for pair in range(mt_per_chunk):
                    ga, gb = 2 * pair, 2 * pair + 1
                    pss = [
                        pspool.tile(
                            [P, N_CHUNKS[i][1]], dt.float32,
                            tag=f"ps{i}", name=f"ps{i}",
                        )
                        for _ in range(2)
                    ]
                    for k in range(KT):
                        for g, ps in zip((ga, gb), pss):
                            xt = xts0 if g < mt_per_chunk else xts1
                            do_mm(ps, xt, g % mt_per_chunk, k, i)
                    for g, ps in zip((ga, gb), pss):
                        epilogue(ps, g * P, i)

            phase(0)
            unpack_group(1, qts1, nc.vector, upool)
            qts2 = load_group_qt(2, u2pool, "2")
            phase(1)
            unpack_group(2, qts2, nc.vector, u2pool, "2")
            phase(2)

            # ---- steady state ----
            for mc in range(2, n_mchunks):
                xts = load_chunk(mc)
                for mt in range(mt_per_chunk):
                    pss = [
                        pspool.tile(
                            [P, wd], dt.float32, tag=f"ps{i}", name=f"ps{i}"
                        )
                        for i, (o, wd) in enumerate(N_CHUNKS)
                    ]
                    for k in range(KT):
                        for i in range(len(N_CHUNKS)):
                            do_mm(pss[i], xts, mt, k, i)
                    for i in range(len(N_CHUNKS)):
                        epilogue(pss[i], mc * M_CHUNK + mt * P, i)

    nc.compile()
    return nc


_SHAMT16 = (4 * (np.arange(P, dtype=np.int16) % 4)).reshape(P, 1)


def _qw_int16(qw_slice):
    """Reinterpret packed int32 nibbles as int16 rows (layout-only) and
    row-replicate 4x so SBUF partition p holds the packed word for k-row p."""
    a = np.ascontiguousarray(qw_slice)  # [512, nsh] int32
    rows, cols = a.shape
    a16 = (
        a.view(np.int16)
        .reshape(rows, cols, 2)
        .transpose(0, 2, 1)
        .reshape(rows * 2, cols)
    )  # row 2r = low 4 nibbles of packed row r, 2r+1 = high 4
    return np.ascontiguousarray(np.repeat(a16, 4, axis=0))


def make_in_maps(x2d, qweight, scales, zeros, bias):
    """Per-core input maps (host-side sharding / layout prep only)."""
    in_maps = []
    for c in range(NCORES):
        sl = slice(c * NSH, (c + 1) * NSH)
        in_maps.append(
            {
                "x": x2d,
                "qw": _qw_int16(qweight[:, sl]),
                "scales": np.ascontiguousarray(scales[sl, 0]).astype(
                    ml_dtypes.bfloat16
                ),
                "zeros": np.ascontiguousarray(zeros[sl, 0]).astype(
                    ml_dtypes.bfloat16
                ),
                "bias": np.ascontiguousarray(bias[sl]).astype(ml_dtypes.bfloat16),
                "shamt": _SHAMT16,
            }
        )
    return in_maps


_NC_CACHE = {}


def _get_nc(m_rows):
    if m_rows not in _NC_CACHE:
        _NC_CACHE[m_rows] = build(m_rows)
    return _NC_CACHE[m_rows]


def run_spmd(x2d, qweight, scales, zeros, bias, trace=False, **kwargs):
    """Run on the 8 NeuronCores; returns (out2d [8192, 11008] f32, results)."""
    from concourse.bass_utils import run_bass_kernel_spmd

    m_rows = x2d.shape[0]
    nc = _get_nc(m_rows)
    in_maps = make_in_maps(x2d, qweight, scales, zeros, bias)
    res = run_bass_kernel_spmd(
        nc, in_maps, list(range(NCORES)), trace=trace, **kwargs
    )
    outs = [res.results[c]["out"] for c in range(NCORES)]
    out2d = np.concatenate(outs, axis=1)
    return out2d, res


def kernel(x, qweight, scales, zeros, bias):
    x = np.asarray(x, dtype=np.float32)
    qweight = np.asarray(qweight, dtype=np.int32)
    scales = np.asarray(scales, dtype=np.float32)
    zeros = np.asarray(zeros, dtype=np.float32)
    bias = np.asarray(bias, dtype=np.float32)

    b, s, k_in = x.shape
    x2d = np.ascontiguousarray(x.reshape(b * s, k_in))
    out2d, _ = run_spmd(x2d, qweight, scales, zeros, bias)
    return out2d.reshape(b, s, OUT)
